# revision 51
# baseline (speedup 1.0000x reference)
"""Causal multi-head attention (b=2, n=2048, dim=1024, 16 heads) on 8 trn2
NeuronCores.

Sharding: core j = 4*g + r owns batch g and heads 4r..4r+3 (tensor parallel
over heads within each batch's 4-core group). Each core:
  P1  projects q/k (transposed layout [head_dim, tokens]) and v (natural
      [tokens, head_dim], ones-augmented) for its 4 heads from x^T. All
      matmul operands are bf16 (fp32 PSUM accumulate); rel-err budget is
      2e-2 so bf16 inputs are comfortably inside it and halve DMA + SBUF
      traffic while enabling fast-weight-load on the PE.
  P2  causal attention per head pair in S^T orientation, software-pipelined:
      slot j emits S^T(j) then AV(j-1), so the exp(j) on the scalar engine
      overlaps AV(j-1) + S^T(j+1) on the PE instead of stalling it.
      S^T = two row-group-concurrent matmuls (d=64 contraction each), exp
      without max subtraction (scores are O(1) here), triangular mask on
      diagonal tiles applied on gpsimd off the PE/ACT critical path,
      O'^T = V_aug.T @ expS^T accumulated in PSUM (row 64 = softmax
      denominator Z). Normalization reads PSUM directly: approx-fast
      reciprocal of the Z rows (51 ULP), DRAM-bounce broadcast, scale muls
      split across vector/gpsimd writing bf16 A2A staging.
  A2A transposes the sharding of A^T = [head_dim*heads, tokens] from
      head-sharded to token-sharded (8-core AllToAll on bf16 payload; each
      core addresses its group's chunks via partition_id-derived offsets).
  P3  out = A^T.T @ Wout for this core's 512-token block, plus biases.
Host: transposes x per batch, slices weights per head group, converts the
matmul path to bf16, gathers the 8 [512, 1024] row blocks into the full
[2, 2048, 1024] fp32 output.
"""
import numpy as np
from ml_dtypes import bfloat16

import concourse.bass as bass
import concourse.mybir as mybir
import concourse.tile as tile
from concourse.bass import AP, ds
from concourse.bass_utils import run_bass_kernel_spmd
from concourse.vector_clock import ScopedClock

F32 = mybir.dt.float32
BF16 = mybir.dt.bfloat16
EXP = mybir.ActivationFunctionType.Exp

N_CORES = 8
B, N, DIM, H = 2, 2048, 1024, 16
D = DIM // H                 # 64
HL = 4                       # heads per core
KT = DIM // 128              # 8 contraction k-tiles
NJ = N // 128                # 16 key tiles per batch
NI = N // 512                # 4 query i-blocks per batch
SCALE = float(D) ** -0.5


def _split_multi_waits(nc):
    """This walrus build rejects instructions carrying more than one sync
    wait. Hoist extra waits onto same-engine NoOps inserted directly before
    the offending instruction (engines execute their stream in order, so
    this preserves semantics)."""
    n = 0
    for f in nc.m.functions:
        for bb in f.blocks:
            insts = bb.instructions
            out = []
            changed = False
            for inst in insts:
                si = inst.sync_info
                waits = list(si.on_wait) if si is not None and si.on_wait else []
                if len(waits) > 1:
                    changed = True
                    for w in waits[:-1]:
                        nop = mybir.InstNoOp(name=f"I-waitfix-{n}", ins=[],
                                             outs=[])
                        n += 1
                        nop.engine = inst.engine
                        nop.sync_info = mybir.SyncInfo(on_wait=[w],
                                                       on_update=[])
                        out.append(nop)
                    si.on_wait = waits[-1:]
                out.append(inst)
            if changed:
                insts[:] = out
    return n


class _TC(tile.TileContext):
    """Tail drain in this walrus build only supports one sync-wait per CTRL
    instruction; spread the residual global-clock waits over SP nops, and
    split any remaining multi-wait instructions after scheduling."""

    def _drain_and_barrier(self, tick_clock, wait_clock):
        nop = self.nc.sync.nop()
        wait_clock.add_sem_waits(nop.ins, ScopedClock({None: tick_clock.global_clock}))
        si = nop.ins.sync_info
        waits = list(si.on_wait or []) if si is not None else []
        if len(waits) > 1:
            si.on_wait = waits[:1]
            for w in waits[1:]:
                extra = self.nc.sync.nop()
                extra.ins.sync_info = mybir.SyncInfo(on_wait=[w], on_update=[])
        self.nc.sync.drain()
        self.nc.all_engine_barrier()
        assert self.sems is not None
        popped = self.nc._tile_sem_poison_stack.pop()
        assert popped is self._sem_poison
        self.nc.clear_and_free_semaphores(list(self.sems.allocated().values()))
        self.nc.all_engine_barrier()

    def __exit__(self, exc_type, exc_val, exc_tb):
        r = super().__exit__(exc_type, exc_val, exc_tb)
        if exc_type is None:
            _split_multi_waits(self.nc)
        return r


def _bcast(src_dram_row, parts):
    """DRAM [1, n] row -> AP replicating it over `parts` partitions (step-0
    leading dim; only legal for DRAM sources)."""
    return AP(src_dram_row.tensor, src_dram_row.offset,
              [[0, parts]] + list(src_dram_row.ap)[1:])


def _build():
    nc = bass.Bass(trn_type="TRN2", target_bir_lowering=False, debug=False,
                   num_devices=N_CORES)
    dt = F32
    # pre-tiled on host: [128, KT*width] rows are fully linear so the bulk
    # DMAs run at line rate instead of 1KB-descriptor rate
    xt_d = nc.dram_tensor("xt", [128, KT * N], BF16, kind="ExternalInput").ap()
    wq_d = nc.dram_tensor("wq", [128, KT * HL * D], BF16, kind="ExternalInput").ap()
    wk_d = nc.dram_tensor("wk", [128, KT * HL * D], BF16, kind="ExternalInput").ap()
    wv_d = nc.dram_tensor("wv", [128, KT * HL * D], BF16, kind="ExternalInput").ap()
    wout_d = nc.dram_tensor("wout", [128, KT * DIM], BF16, kind="ExternalInput").ap()
    bq_d = nc.dram_tensor("bq", [HL * D, 1], dt, kind="ExternalInput").ap()
    bk_d = nc.dram_tensor("bk", [HL * D, 1], dt, kind="ExternalInput").ap()
    bv_d = nc.dram_tensor("bv", [1, HL * D], dt, kind="ExternalInput").ap()
    bout_d = nc.dram_tensor("bout", [1, DIM], dt, kind="ExternalInput").ap()
    mask_d = nc.dram_tensor("mask", [128, 128], BF16, kind="ExternalInput").ap()
    ones_d = nc.dram_tensor("ones", [1, HL], BF16, kind="ExternalInput").ap()
    sel_d = nc.dram_tensor("sel", [8, 4 * 128], BF16, kind="ExternalInput").ap()
    out_d = nc.dram_tensor("out", [N // HL, DIM], dt, kind="ExternalOutput").ap()

    with _TC(nc) as tc, \
            nc.allow_low_precision(reason="bf16 matmul operand staging"):
        _body(nc, tc, xt_d, wq_d, wk_d, wv_d, wout_d, bq_d, bk_d, bv_d,
              bout_d, mask_d, ones_d, sel_d, out_d)
    return nc


def _body(nc, tc, xt_d, wq_d, wk_d, wv_d, wout_d, bq_d, bk_d, bv_d, bout_d,
          mask_d, ones_d, sel_d, out_d):
    mm = nc.tensor.matmul
    with tc.tile_pool(name="persist", bufs=1) as pers:
        # Persistent SBUF: q^T/k^T per head pair, v (ones-augmented) per
        # 128-token tile, A^T per head pair, mask, biases.
        qt = [pers.tile([128, N], BF16, tag=f"qt{p}", name=f"qt{p}") for p in (0, 1)]
        kt = [pers.tile([128, N], BF16, tag=f"kt{p}", name=f"kt{p}") for p in (0, 1)]
        VS = 96  # per-head stride in v tiles: 192B keeps lhsT 128B-aligned
        vt = [pers.tile([128, HL * VS], BF16, tag=f"v{t}", name=f"v{t}")
              for t in range(NJ)]
        mask_sb = pers.tile([128, 128], BF16, tag="mask", name="mask_sb")
        bqc = pers.tile([128, 2], F32, tag="bqc", name="bqc")
        bkc = pers.tile([128, 2], F32, tag="bkc", name="bkc")
        bvb = pers.tile([128, HL * D], F32, tag="bvb", name="bvb")
        boutb = pers.tile([128, DIM], F32, tag="boutb", name="boutb")

        ones_sb = pers.tile([128, HL], BF16, tag="ones", name="ones_sb")
        # selector for the receiver-side 1/Z broadcast: rzp[:, k, :] =
        # sel[:, k, :].T @ rzq picks Z-row (p//64)*4+k for out partition p
        sel_sb = pers.tile([8, 4, 128], BF16, tag="sel", name="sel_sb")

        # -------- P1 + P2 pair 0, interleaved in causal waves --------
        # Input DMA is the startup bottleneck (3 dynamic queues at ~37 GB/s
        # each), so x is staged token-major and consumed in four 512-token
        # waves: wave Q projects q/k/v for its token block, then immediately
        # runs pair-0 attention for query block Q (causality means only
        # tokens <= 512(Q+1) are needed). The scalar engine's exp stream —
        # the P2 bottleneck — starts ~15us in instead of after all of P1.
        # multi-engine partition id so gsel-addressed DMAs can issue from
        # any DMA-capable queue, not just SP
        pid = nc.partition_id()
        gsel = nc.snap(pid // 4, min_val=0, max_val=1)
        with (tc.tile_pool(name="p1s", bufs=1) as p1s,
              tc.tile_pool(name="p3w", bufs=1) as p3w,
              tc.tile_pool(name="p2s", bufs=4) as p2s,
              tc.tile_pool(name="p2n", bufs=3) as p2n,
              tc.tile_pool(name="dram", bufs=1, space="DRAM") as dram):
            w_sb = {}
            for nm in ("wq", "wk", "wv"):
                w_sb[nm] = p1s.tile([128, KT, HL * D], BF16, tag=nm,
                                    name=f"{nm}_sb")
            xt_sb = p1s.tile([128, NI, KT, 512], BF16, tag="xt", name="xt_sb")
            wout_sb = p3w.tile([128, KT, DIM], BF16, tag="wout",
                               name="wout_sb")
            # tiny constants first (needed by wave 0's drains/masks)
            nc.gpsimd.dma_start(mask_sb[:], mask_d[:])
            nc.gpsimd.dma_start(bqc[:],
                                bq_d.rearrange("(m p) o -> p (m o)", p=128))
            nc.gpsimd.dma_start(bkc[:],
                                bk_d.rearrange("(m p) o -> p (m o)", p=128))
            nc.gpsimd.dma_start(bvb[:], _bcast(bv_d[0:1, :], 128))
            nc.gpsimd.dma_start(boutb[:], _bcast(bout_d[0:1, :], 128))
            nc.gpsimd.dma_start(ones_sb[:], _bcast(ones_d[0:1, :], 128))
            nc.gpsimd.dma_start(sel_sb[:],
                                sel_d.rearrange("p (k e) -> p k e", k=4))
            # bulk input plan, in consumption order, split into ~256-512KB
            # chunks spread over the three queues so wave 0's operands land
            # in ~7us and later waves stream in behind the compute
            xt_dv = xt_d.rearrange("p (q k n) -> p q k n", q=NI, k=KT)
            wq_dv = wq_d.rearrange("p (k e) -> p k e", k=KT)
            wk_dv = wk_d.rearrange("p (k e) -> p k e", k=KT)
            wv_dv = wv_d.rearrange("p (k e) -> p k e", k=KT)
            plan = [
                (nc.scalar, w_sb["wq"][:, 0:4, :], wq_dv[:, 0:4, :]),
                (nc.sync, xt_sb[:, 0, 0:4, :], xt_dv[:, 0, 0:4, :]),
                (nc.gpsimd, xt_sb[:, 0, 4:8, :], xt_dv[:, 0, 4:8, :]),
                (nc.scalar, w_sb["wq"][:, 4:8, :], wq_dv[:, 4:8, :]),
                (nc.sync, w_sb["wk"][:, 0:4, :], wk_dv[:, 0:4, :]),
                (nc.gpsimd, w_sb["wk"][:, 4:8, :], wk_dv[:, 4:8, :]),
                (nc.scalar, w_sb["wv"][:], wv_dv),
                (nc.sync, xt_sb[:, 1, 0:4, :], xt_dv[:, 1, 0:4, :]),
                (nc.gpsimd, xt_sb[:, 1, 4:8, :], xt_dv[:, 1, 4:8, :]),
                (nc.scalar, xt_sb[:, 2, 0:4, :], xt_dv[:, 2, 0:4, :]),
                (nc.sync, xt_sb[:, 2, 4:8, :], xt_dv[:, 2, 4:8, :]),
                (nc.gpsimd, xt_sb[:, 3, 0:4, :], xt_dv[:, 3, 0:4, :]),
                (nc.scalar, xt_sb[:, 3, 4:8, :], xt_dv[:, 3, 4:8, :]),
            ]
            for eng, dst_ap, src_ap in plan:
                eng.dma_start(dst_ap, src_ap)

            CR = 130
            a2a_in = [dram.tile([8 * CR, 512], BF16, name=f"a2a_in{h}")
                      for h in (0, 1)]
            a2a_out = [dram.tile([8 * CR, 512], BF16, name=f"a2a_out{h}")
                       for h in (0, 1)]
            # PSUM during the wave phase: qk 1 + v 1 + scores 4 + O' 2 = 8
            psum_ctx = (tc.tile_pool(name="qkp", bufs=1, space="PSUM"),
                        tc.tile_pool(name="pvp", bufs=1, space="PSUM"),
                        tc.tile_pool(name="sp", bufs=2, space="PSUM"),
                        tc.tile_pool(name="op", bufs=1, space="PSUM"))
            qkp, pvp, sp, op = [c.__enter__() for c in psum_ctx]

            def attention_block(pp, I):
                """Pair pp, query block I: software-pipelined S^T / exp /
                AV; ships unnormalized O' + Z rows to the A2A staging
                buffer (chunk = 130 rows; 8-core AllToAll with the
                4-core-group duplication trick)."""
                i0 = 512 * I
                last = 4 * I + 3
                poA = op.tile([D + 1, 512], F32, tag="oA", name="poA")
                poB = op.tile([D + 1, 512], F32, tag="oB", name="poB")

                def emit_av(e, f0, jj):
                    vv = vt[jj].rearrange("p (h x) -> p h x", x=VS)
                    mm(poA[:, f0:512], vv[:, 2 * pp, 0:D + 1],
                       e[:, f0:512],
                       start=(jj == 0), stop=(jj == last))
                    mm(poB[:, f0:512], vv[:, 2 * pp + 1, 0:D + 1],
                       e[:, 512 + f0:1024],
                       start=(jj == 0), stop=(jj == last))

                pend = None
                for jj in range(4 * I + 4):
                    di = jj - 4 * I
                    f0 = 128 * di if di >= 0 else 0
                    ps = sp.tile([128, 1024], F32, tag="s", name="ps_s")
                    # explicit row-group tile positions: the two heads'
                    # d=64-contraction matmuls occupy disjoint halves of
                    # the PE array and run concurrently
                    mm(ps[:, f0:512],
                       kt[pp][0:64, 128 * jj:128 * jj + 128],
                       qt[pp][0:64, i0 + f0:i0 + 512],
                       start=True, stop=True, tile_position=(0, 0))
                    mm(ps[:, 512 + f0:1024],
                       kt[pp][64:128, 128 * jj:128 * jj + 128],
                       qt[pp][64:128, i0 + f0:i0 + 512],
                       start=True, stop=True, tile_position=(64, 0))
                    e = p2s.tile([128, 1024], BF16, tag="e", name="e_s")
                    ev = e.rearrange("p (h x) -> p h x", x=512)
                    pv2 = ps.rearrange("p (h x) -> p h x", x=512)
                    nc.scalar.activation(ev[:, :, f0:512],
                                         pv2[:, :, f0:512], EXP,
                                         scale=SCALE)
                    if di >= 0:
                        nc.vector.tensor_mul(
                            ev[:, :, f0:f0 + 128],
                            ev[:, :, f0:f0 + 128],
                            mask_sb[:, None, :].to_broadcast((128, 2, 128)))
                    if pend is not None:
                        emit_av(*pend)
                    pend = (e, f0, jj)
                emit_av(*pend)
                stA = p2n.tile([65, 512], BF16, tag="stA", name="stA")
                stB = p2n.tile([65, 512], BF16, tag="stB", name="stB")
                nc.vector.tensor_copy(stA[:], poA[:])
                nc.vector.tensor_copy(stB[:], poB[:])
                for gg in (0, 1):
                    r0 = CR * (4 * gg + I)
                    nc.sync.dma_start(a2a_in[pp][r0:r0 + 64, :], stA[0:64, :])
                    nc.sync.dma_start(a2a_in[pp][r0 + 64:r0 + 128, :],
                                      stB[0:64, :])
                    nc.sync.dma_start(a2a_in[pp][r0 + 128:r0 + 129, :],
                                      stA[64:65, :])
                    nc.sync.dma_start(a2a_in[pp][r0 + 129:r0 + 130, :],
                                      stB[64:65, :])

            for Q in range(NI):
                # projections for token block Q (all 4 heads)
                for w, bcol, dst in (("wq", bqc, qt), ("wk", bkc, kt)):
                    for mt in (0, 1):
                        ps = qkp.tile([128, 512], F32, tag="pqk",
                                      name="ps_qk")
                        for kk in range(KT):
                            mm(ps[:],
                               w_sb[w][:, kk, 128 * mt:128 * mt + 128],
                               xt_sb[:, Q, kk, :],
                               start=(kk == 0), stop=(kk == KT - 1))
                        nc.vector.tensor_scalar_add(
                            dst[mt][:, 512 * Q:512 * Q + 512], ps[:],
                            bcol[:, mt:mt + 1])
                for t4 in range(4):
                    tt = 4 * Q + t4
                    ps = pvp.tile([128, HL * D], F32, tag="pv", name="ps_v")
                    for kk in range(KT):
                        mm(ps[:],
                           xt_sb[:, Q, kk, 128 * t4:128 * t4 + 128],
                           w_sb["wv"][:, kk, :],
                           start=(kk == 0), stop=(kk == KT - 1))
                    vv = vt[tt].rearrange("p (h x) -> p h x", x=VS)
                    nc.vector.tensor_add(vv[:, :, 0:D],
                                         ps.rearrange("p (h x) -> p h x",
                                                      x=D),
                                         bvb.rearrange("p (h x) -> p h x",
                                                       x=D))
                    nc.vector.tensor_copy(
                        vv[:, :, D:D + 1],
                        ones_sb.rearrange("p (h o) -> p h o", o=1))
                # pair-0 attention for query block Q rides this wave
                attention_block(0, Q)
            nc.gpsimd.collective_compute(
                "AllToAll", mybir.AluOpType.bypass,
                replica_groups=[list(range(N_CORES))],
                ins=[a2a_in[0].opt()], outs=[a2a_out[0].opt()])
            # wout lands during pair-1 attention, when input queues are
            # idle; split across all three queues so no single queue
            # carries the whole 2MB (the gpsimd queue later serves P3's
            # atf chunks)
            wout_dv = wout_d.rearrange("p (k c) -> p k c", k=KT)
            for i, eng in enumerate((nc.scalar, nc.sync, nc.gpsimd,
                                     nc.scalar)):
                eng.dma_start(wout_sb[:, 2 * i:2 * i + 2, :],
                              wout_dv[:, 2 * i:2 * i + 2, :])
            for Q in range(NI):
                attention_block(1, Q)
            nc.gpsimd.collective_compute(
                "AllToAll", mybir.AluOpType.bypass,
                replica_groups=[list(range(N_CORES))],
                ins=[a2a_in[1].opt()], outs=[a2a_out[1].opt()])
            for c in reversed(psum_ctx):
                c.__exit__(None, None, None)

            # ---------------- P3: output projection ----------------
            # Receiver-side softmax normalization, all on-chip: Z rows from
            # my group's 4 senders land as [8, 512] (multi-lane reciprocal),
            # then tiny ones-column matmuls broadcast 1/Z across the 64
            # head-dim partitions into PSUM, and the A^T tiles are scaled
            # straight from there — no DRAM round trip. The normalize chain
            # for pair 0 runs during the second AllToAll's transfer window.
            with (tc.tile_pool(name="p3s", bufs=2) as p3s,
                  tc.tile_pool(name="p3z", bufs=1, space="PSUM") as p3z,
                  tc.tile_pool(name="p3p", bufs=1, space="PSUM") as p3p):
                atns = []
                eng3 = [nc.sync, nc.scalar]

                def recv_chain(h):
                    """Fetch + normalize pair h's received A^T tiles,
                    pipelined per 128-row chunk: each chunk's DMA rides its
                    own queue and its normalize mul fires as it lands."""
                    av = a2a_out[h].rearrange("(G k p) c -> p G k c", k=4,
                                              p=CR)
                    zq = p3s.tile([8, 512], BF16, tag=f"zq{h}",
                                  name=f"zq{h}", bufs=1)
                    # element order (h2-major, then sender k, then token)
                    # matches zq's flat [8, 512] partition order h2*4+k
                    avz = a2a_out[h].rearrange("(G k p) c -> p k G c",
                                               k=4, p=CR)
                    nc.sync.dma_start(zq[:],
                                      avz[128:130, :, ds(gsel, 1), :])
                    zqf = p3s.tile([8, 512], F32, tag=f"zqf{h}",
                                   name=f"zqf{h}", bufs=1)
                    nc.vector.tensor_copy(zqf[:], zq[:])
                    rzq = p3s.tile([8, 512], F32, tag=f"rzq{h}",
                                   name=f"rzq{h}", bufs=1)
                    nc.vector.reciprocal(rzq[:], zqf[:])
                    rzqb = p3s.tile([8, 512], BF16, tag=f"rzqb{h}",
                                    name=f"rzqb{h}", bufs=1)
                    nc.vector.tensor_copy(rzqb[:], rzq[:])
                    rzp = p3z.tile([128, 4, 512], F32, tag="rzp",
                                   name=f"rzp{h}")
                    atf = p3s.tile([128, 4, 512], BF16, tag=f"atf{h}",
                                   name=f"atf{h}", bufs=1)
                    atn = p3s.tile([128, 4, 512], BF16, tag=f"atn{h}",
                                   name=f"atn{h}", bufs=1)
                    q3 = [nc.scalar, nc.gpsimd, nc.sync, nc.scalar]
                    for k4 in range(4):
                        q3[k4].dma_start(atf[:, k4, :],
                                         av[0:128, ds(gsel, 1), k4, :])
                        mm(rzp[:, k4, :], sel_sb[:, k4, :], rzqb[:],
                           start=True, stop=True)
                        nc.vector.tensor_mul(atn[:, k4, :], atf[:, k4, :],
                                             rzp[:, k4, :])
                    atns.append(atn)

                def pout_mms(pso, it, ct, kks):
                    for kk in kks:
                        h, k4 = divmod(kk, 4)
                        mm(pso[:],
                           atns[h][:, k4, 128 * it:128 * it + 128],
                           wout_sb[:, kk, 512 * ct:512 * ct + 512],
                           start=(kk == 0), stop=(kk == KT - 1))

                def pout_done(pso, it, ct):
                    osb = p3s.tile([128, 512], F32, tag="osb", name="osb")
                    nc.vector.tensor_add(
                        osb[:], pso[:], boutb[:, 512 * ct:512 * ct + 512])
                    eng3[it % 2].dma_start(
                        out_d[128 * it:128 * it + 128,
                              512 * ct:512 * ct + 512], osb[:])

                # pair-0 chunks arrive with the first AllToAll, so their
                # half of the contraction for the first two token tiles
                # runs during the second AllToAll's transfer window
                recv_chain(0)
                pouts = {}
                for it in (0, 1):
                    for ct in (0, 1):
                        pouts[(it, ct)] = p3p.tile(
                            [128, 512], F32, tag=f"po{it}{ct}",
                            name=f"po{it}{ct}", bufs=1)
                        pout_mms(pouts[(it, ct)], it, ct, range(4))
                recv_chain(1)
                # kk-major so each chunk's matmuls start as soon as that
                # chunk has landed and been normalized
                for kk in range(4, 8):
                    for it in (0, 1):
                        for ct in (0, 1):
                            pout_mms(pouts[(it, ct)], it, ct, [kk])
                for it in (0, 1):
                    for ct in (0, 1):
                        pout_done(pouts[(it, ct)], it, ct)
                for it in (2, 3):
                    for ct in (0, 1):
                        pso = p3p.tile([128, 512], F32,
                                       tag=f"po{it - 2}{ct}",
                                       name=f"po{it}{ct}", bufs=1)
                        pout_mms(pso, it, ct, range(8))
                        pout_done(pso, it, ct)


_NC_CACHE = {}

# test-only knobs: set TRACE=True before calling kernel() to profile; the
# BassKernelResults of the last run lands in LAST_RESULT. TRACE_CORES
# optionally selects which cores to profile (e.g. list(range(8))).
TRACE = False
TRACE_CORES = None
LAST_RESULT = None


def _get_nc():
    if "nc" not in _NC_CACHE:
        _NC_CACHE["nc"] = _build()
    return _NC_CACHE["nc"]


def kernel(x, Wq, bq, Wkv, bkv, Wout, bout):
    x = np.asarray(x, np.float32)
    Wq = np.asarray(Wq, np.float32)
    bq = np.asarray(bq, np.float32)
    Wkv = np.asarray(Wkv, np.float32)
    bkv = np.asarray(bkv, np.float32)
    Wout = np.asarray(Wout, np.float32)
    bout = np.asarray(bout, np.float32)

    def ktile(a):  # [128*KT_rows, width] -> [128, KT_rows*width], row-linear
        kk = a.shape[0] // 128
        return np.ascontiguousarray(
            a.reshape(kk, 128, a.shape[1]).transpose(1, 0, 2).reshape(128, -1))

    def xtile(a):  # x^T [1024, 2048] -> [128, (q kk 512)] token-major
        return np.ascontiguousarray(
            a.reshape(KT, 128, NI, 512).transpose(1, 2, 0, 3).reshape(128, -1))

    def tob(a):
        return np.ascontiguousarray(a.astype(bfloat16))

    mask = np.triu(np.ones((128, 128), np.float32))  # mask[p, c] = c >= p
    sel = np.zeros((8, 4, 128), np.float32)
    for p in range(128):
        for k in range(4):
            sel[(p // 64) * 4 + k, k, p] = 1.0
    sel = tob(sel.reshape(8, 512))
    xts = [tob(xtile(np.ascontiguousarray(x[g].T))) for g in range(B)]
    # out-proj contraction row order: collective h, rank kk carries heads
    # (4*kk + 2h, 4*kk + 2h + 1) -> permute Wout rows to match
    wout_perm = np.concatenate(
        [Wout[256 * kk + 128 * h:256 * kk + 128 * h + 128]
         for h in (0, 1) for kk in range(4)])
    wout_t = tob(ktile(wout_perm))
    in_maps = []
    for j in range(N_CORES):
        g, r = divmod(j, 4)
        cols = slice(HL * D * r, HL * D * (r + 1))
        in_maps.append({
            "xt": xts[g],
            "wq": tob(ktile(Wq[:, cols])),
            "wk": tob(ktile(Wkv[:, 0:DIM][:, cols])),
            "wv": tob(ktile(Wkv[:, DIM:2 * DIM][:, cols])),
            "wout": wout_t,
            "bq": np.ascontiguousarray(bq[cols][:, None]),
            "bk": np.ascontiguousarray(bkv[0:DIM][cols][:, None]),
            "bv": np.ascontiguousarray(bkv[DIM:2 * DIM][cols][None, :]),
            "bout": np.ascontiguousarray(bout[None, :]),
            "mask": tob(mask),
            "ones": np.ones((1, HL), bfloat16),
            "sel": sel,
        })
    res = run_bass_kernel_spmd(_get_nc(), in_maps, list(range(N_CORES)),
                               trace=TRACE, trace_cores=TRACE_CORES)
    global LAST_RESULT
    LAST_RESULT = res
    out = np.empty((B, N, DIM), np.float32)
    for j in range(N_CORES):
        g, r = divmod(j, 4)
        out[g, 512 * r:512 * (r + 1)] = res.results[j]["out"]
    return out


# revision 53
# speedup vs baseline: 1.0055x; 1.0055x over previous
"""Causal multi-head attention (b=2, n=2048, dim=1024, 16 heads) on 8 trn2
NeuronCores.

Sharding: core j = 4*g + r owns batch g and heads 4r..4r+3 (tensor parallel
over heads within each batch's 4-core group). Each core:
  P1  projects q/k (transposed layout [head_dim, tokens]) and v (natural
      [tokens, head_dim], ones-augmented) for its 4 heads from x^T. All
      matmul operands are bf16 (fp32 PSUM accumulate); rel-err budget is
      2e-2 so bf16 inputs are comfortably inside it and halve DMA + SBUF
      traffic while enabling fast-weight-load on the PE.
  P2  causal attention per head pair in S^T orientation, software-pipelined:
      slot j emits S^T(j) then AV(j-1), so the exp(j) on the scalar engine
      overlaps AV(j-1) + S^T(j+1) on the PE instead of stalling it.
      S^T = two row-group-concurrent matmuls (d=64 contraction each), exp
      without max subtraction (scores are O(1) here), triangular mask on
      diagonal tiles applied on gpsimd off the PE/ACT critical path,
      O'^T = V_aug.T @ expS^T accumulated in PSUM (row 64 = softmax
      denominator Z). Normalization reads PSUM directly: approx-fast
      reciprocal of the Z rows (51 ULP), DRAM-bounce broadcast, scale muls
      split across vector/gpsimd writing bf16 A2A staging.
  A2A transposes the sharding of A^T = [head_dim*heads, tokens] from
      head-sharded to token-sharded (8-core AllToAll on bf16 payload; each
      core addresses its group's chunks via partition_id-derived offsets).
  P3  out = A^T.T @ Wout for this core's 512-token block, plus biases.
Host: transposes x per batch, slices weights per head group, converts the
matmul path to bf16, gathers the 8 [512, 1024] row blocks into the full
[2, 2048, 1024] fp32 output.
"""
import numpy as np
from ml_dtypes import bfloat16

import concourse.bass as bass
import concourse.mybir as mybir
import concourse.tile as tile
from concourse.bass import AP, ds
from concourse.bass_utils import run_bass_kernel_spmd
from concourse.vector_clock import ScopedClock

F32 = mybir.dt.float32
BF16 = mybir.dt.bfloat16
EXP = mybir.ActivationFunctionType.Exp

N_CORES = 8
B, N, DIM, H = 2, 2048, 1024, 16
D = DIM // H                 # 64
HL = 4                       # heads per core
KT = DIM // 128              # 8 contraction k-tiles
NJ = N // 128                # 16 key tiles per batch
NI = N // 512                # 4 query i-blocks per batch
SCALE = float(D) ** -0.5


def _split_multi_waits(nc):
    """This walrus build rejects instructions carrying more than one sync
    wait. Hoist extra waits onto same-engine NoOps inserted directly before
    the offending instruction (engines execute their stream in order, so
    this preserves semantics)."""
    n = 0
    for f in nc.m.functions:
        for bb in f.blocks:
            insts = bb.instructions
            out = []
            changed = False
            for inst in insts:
                si = inst.sync_info
                waits = list(si.on_wait) if si is not None and si.on_wait else []
                if len(waits) > 1:
                    changed = True
                    for w in waits[:-1]:
                        nop = mybir.InstNoOp(name=f"I-waitfix-{n}", ins=[],
                                             outs=[])
                        n += 1
                        nop.engine = inst.engine
                        nop.sync_info = mybir.SyncInfo(on_wait=[w],
                                                       on_update=[])
                        out.append(nop)
                    si.on_wait = waits[-1:]
                out.append(inst)
            if changed:
                insts[:] = out
    return n


class _TC(tile.TileContext):
    """Tail drain in this walrus build only supports one sync-wait per CTRL
    instruction; spread the residual global-clock waits over SP nops, and
    split any remaining multi-wait instructions after scheduling."""

    def _drain_and_barrier(self, tick_clock, wait_clock):
        nop = self.nc.sync.nop()
        wait_clock.add_sem_waits(nop.ins, ScopedClock({None: tick_clock.global_clock}))
        si = nop.ins.sync_info
        waits = list(si.on_wait or []) if si is not None else []
        if len(waits) > 1:
            si.on_wait = waits[:1]
            for w in waits[1:]:
                extra = self.nc.sync.nop()
                extra.ins.sync_info = mybir.SyncInfo(on_wait=[w], on_update=[])
        self.nc.sync.drain()
        self.nc.all_engine_barrier()
        assert self.sems is not None
        popped = self.nc._tile_sem_poison_stack.pop()
        assert popped is self._sem_poison
        self.nc.clear_and_free_semaphores(list(self.sems.allocated().values()))
        self.nc.all_engine_barrier()

    def __exit__(self, exc_type, exc_val, exc_tb):
        r = super().__exit__(exc_type, exc_val, exc_tb)
        if exc_type is None:
            _split_multi_waits(self.nc)
        return r


def _bcast(src_dram_row, parts):
    """DRAM [1, n] row -> AP replicating it over `parts` partitions (step-0
    leading dim; only legal for DRAM sources)."""
    return AP(src_dram_row.tensor, src_dram_row.offset,
              [[0, parts]] + list(src_dram_row.ap)[1:])


def _build():
    nc = bass.Bass(trn_type="TRN2", target_bir_lowering=False, debug=False,
                   num_devices=N_CORES)
    dt = F32
    # pre-tiled on host: [128, KT*width] rows are fully linear so the bulk
    # DMAs run at line rate instead of 1KB-descriptor rate
    xt_d = nc.dram_tensor("xt", [128, KT * N], BF16, kind="ExternalInput").ap()
    wq_d = nc.dram_tensor("wq", [128, KT * HL * D], BF16, kind="ExternalInput").ap()
    wk_d = nc.dram_tensor("wk", [128, KT * HL * D], BF16, kind="ExternalInput").ap()
    wv_d = nc.dram_tensor("wv", [128, KT * HL * D], BF16, kind="ExternalInput").ap()
    wout_d = nc.dram_tensor("wout", [128, KT * DIM], BF16, kind="ExternalInput").ap()
    bq_d = nc.dram_tensor("bq", [HL * D, 1], dt, kind="ExternalInput").ap()
    bk_d = nc.dram_tensor("bk", [HL * D, 1], dt, kind="ExternalInput").ap()
    bv_d = nc.dram_tensor("bv", [1, HL * D], dt, kind="ExternalInput").ap()
    bout_d = nc.dram_tensor("bout", [1, DIM], dt, kind="ExternalInput").ap()
    mask_d = nc.dram_tensor("mask", [128, 128], BF16, kind="ExternalInput").ap()
    ones_d = nc.dram_tensor("ones", [1, HL], BF16, kind="ExternalInput").ap()
    sel_d = nc.dram_tensor("sel", [8, 4 * 128], BF16, kind="ExternalInput").ap()
    out_d = nc.dram_tensor("out", [N // HL, DIM], dt, kind="ExternalOutput").ap()

    with _TC(nc) as tc, \
            nc.allow_low_precision(reason="bf16 matmul operand staging"):
        _body(nc, tc, xt_d, wq_d, wk_d, wv_d, wout_d, bq_d, bk_d, bv_d,
              bout_d, mask_d, ones_d, sel_d, out_d)
    return nc


def _body(nc, tc, xt_d, wq_d, wk_d, wv_d, wout_d, bq_d, bk_d, bv_d, bout_d,
          mask_d, ones_d, sel_d, out_d):
    mm = nc.tensor.matmul
    with tc.tile_pool(name="persist", bufs=1) as pers:
        # Persistent SBUF: q^T/k^T per head pair, v (ones-augmented) per
        # 128-token tile, A^T per head pair, mask, biases.
        qt = [pers.tile([128, N], BF16, tag=f"qt{p}", name=f"qt{p}") for p in (0, 1)]
        kt = [pers.tile([128, N], BF16, tag=f"kt{p}", name=f"kt{p}") for p in (0, 1)]
        VS = 96  # per-head stride in v tiles: 192B keeps lhsT 128B-aligned
        vt = [pers.tile([128, HL * VS], BF16, tag=f"v{t}", name=f"v{t}")
              for t in range(NJ)]
        mask_sb = pers.tile([128, 128], BF16, tag="mask", name="mask_sb")
        bqc = pers.tile([128, 2], F32, tag="bqc", name="bqc")
        bkc = pers.tile([128, 2], F32, tag="bkc", name="bkc")
        bvb = pers.tile([128, HL * D], F32, tag="bvb", name="bvb")
        boutb = pers.tile([128, DIM], F32, tag="boutb", name="boutb")

        ones_sb = pers.tile([128, HL], BF16, tag="ones", name="ones_sb")
        # selector for the receiver-side 1/Z broadcast: rzp[:, k, :] =
        # sel[:, k, :].T @ rzq picks Z-row (p//64)*4+k for out partition p
        sel_sb = pers.tile([8, 4, 128], BF16, tag="sel", name="sel_sb")

        # -------- P1 + P2 pair 0, interleaved in causal waves --------
        # Input DMA is the startup bottleneck (3 dynamic queues at ~37 GB/s
        # each), so x is staged token-major and consumed in four 512-token
        # waves: wave Q projects q/k/v for its token block, then immediately
        # runs pair-0 attention for query block Q (causality means only
        # tokens <= 512(Q+1) are needed). The scalar engine's exp stream —
        # the P2 bottleneck — starts ~15us in instead of after all of P1.
        # multi-engine partition id so gsel-addressed DMAs can issue from
        # any DMA-capable queue, not just SP
        pid = nc.partition_id()
        gsel = nc.snap(pid // 4, min_val=0, max_val=1)
        with (tc.tile_pool(name="p1s", bufs=1) as p1s,
              tc.tile_pool(name="p3w", bufs=1) as p3w,
              tc.tile_pool(name="p2s", bufs=4) as p2s,
              tc.tile_pool(name="p2n", bufs=3) as p2n,
              tc.tile_pool(name="dram", bufs=1, space="DRAM") as dram):
            w_sb = {}
            for nm in ("wq", "wk", "wv"):
                w_sb[nm] = p1s.tile([128, KT, HL * D], BF16, tag=nm,
                                    name=f"{nm}_sb")
            xt_sb = p1s.tile([128, NI, KT, 512], BF16, tag="xt", name="xt_sb")
            wout_sb = p3w.tile([128, KT, DIM], BF16, tag="wout",
                               name="wout_sb")
            # bulk input plan, in consumption order, split into ~256-512KB
            # chunks spread over the three queues so wave 0's operands land
            # in ~7us and later waves stream in behind the compute
            xt_dv = xt_d.rearrange("p (q k n) -> p q k n", q=NI, k=KT)
            wq_dv = wq_d.rearrange("p (k e) -> p k e", k=KT)
            wk_dv = wk_d.rearrange("p (k e) -> p k e", k=KT)
            wv_dv = wv_d.rearrange("p (k e) -> p k e", k=KT)
            plan = [
                (nc.scalar, w_sb["wq"][:, 0:4, :], wq_dv[:, 0:4, :]),
                (nc.sync, xt_sb[:, 0, 0:4, :], xt_dv[:, 0, 0:4, :]),
                (nc.gpsimd, xt_sb[:, 0, 4:8, :], xt_dv[:, 0, 4:8, :]),
                (nc.scalar, w_sb["wq"][:, 4:8, :], wq_dv[:, 4:8, :]),
                (nc.sync, w_sb["wk"][:, 0:4, :], wk_dv[:, 0:4, :]),
                (nc.gpsimd, w_sb["wk"][:, 4:8, :], wk_dv[:, 4:8, :]),
                (nc.scalar, w_sb["wv"][:], wv_dv),
                (nc.sync, xt_sb[:, 1, 0:4, :], xt_dv[:, 1, 0:4, :]),
                (nc.gpsimd, xt_sb[:, 1, 4:8, :], xt_dv[:, 1, 4:8, :]),
                (nc.scalar, xt_sb[:, 2, 0:4, :], xt_dv[:, 2, 0:4, :]),
                (nc.sync, xt_sb[:, 2, 4:8, :], xt_dv[:, 2, 4:8, :]),
                (nc.gpsimd, xt_sb[:, 3, 0:4, :], xt_dv[:, 3, 0:4, :]),
                (nc.scalar, xt_sb[:, 3, 4:8, :], xt_dv[:, 3, 4:8, :]),
            ]
            for eng, dst_ap, src_ap in plan:
                eng.dma_start(dst_ap, src_ap)
            # tiny constants trail the bulk issues (the serial SWDGE issue
            # cost would otherwise delay wave 0's x chunk by ~8us); they
            # still land well before their first uses
            nc.gpsimd.dma_start(mask_sb[:], mask_d[:])
            nc.gpsimd.dma_start(bqc[:],
                                bq_d.rearrange("(m p) o -> p (m o)", p=128))
            nc.gpsimd.dma_start(bkc[:],
                                bk_d.rearrange("(m p) o -> p (m o)", p=128))
            nc.gpsimd.dma_start(bvb[:], _bcast(bv_d[0:1, :], 128))
            nc.gpsimd.dma_start(boutb[:], _bcast(bout_d[0:1, :], 128))
            nc.gpsimd.dma_start(ones_sb[:], _bcast(ones_d[0:1, :], 128))
            nc.gpsimd.dma_start(sel_sb[:],
                                sel_d.rearrange("p (k e) -> p k e", k=4))

            CR = 130
            a2a_in = [dram.tile([8 * CR, 512], BF16, name=f"a2a_in{h}")
                      for h in (0, 1)]
            a2a_out = [dram.tile([8 * CR, 512], BF16, name=f"a2a_out{h}")
                       for h in (0, 1)]
            # PSUM during the wave phase: qk 1 + v 1 + scores 4 + O' 2 = 8
            psum_ctx = (tc.tile_pool(name="qkp", bufs=1, space="PSUM"),
                        tc.tile_pool(name="pvp", bufs=1, space="PSUM"),
                        tc.tile_pool(name="sp", bufs=2, space="PSUM"),
                        tc.tile_pool(name="op", bufs=1, space="PSUM"))
            qkp, pvp, sp, op = [c.__enter__() for c in psum_ctx]

            def attention_block(pp, I):
                """Pair pp, query block I: software-pipelined S^T / exp /
                AV; ships unnormalized O' + Z rows to the A2A staging
                buffer (chunk = 130 rows; 8-core AllToAll with the
                4-core-group duplication trick)."""
                i0 = 512 * I
                last = 4 * I + 3
                poA = op.tile([D + 1, 512], F32, tag="oA", name="poA")
                poB = op.tile([D + 1, 512], F32, tag="oB", name="poB")

                def emit_av(e, f0, jj):
                    vv = vt[jj].rearrange("p (h x) -> p h x", x=VS)
                    mm(poA[:, f0:512], vv[:, 2 * pp, 0:D + 1],
                       e[:, f0:512],
                       start=(jj == 0), stop=(jj == last))
                    mm(poB[:, f0:512], vv[:, 2 * pp + 1, 0:D + 1],
                       e[:, 512 + f0:1024],
                       start=(jj == 0), stop=(jj == last))

                pend = None
                for jj in range(4 * I + 4):
                    di = jj - 4 * I
                    f0 = 128 * di if di >= 0 else 0
                    ps = sp.tile([128, 1024], F32, tag="s", name="ps_s")
                    # explicit row-group tile positions: the two heads'
                    # d=64-contraction matmuls occupy disjoint halves of
                    # the PE array and run concurrently
                    mm(ps[:, f0:512],
                       kt[pp][0:64, 128 * jj:128 * jj + 128],
                       qt[pp][0:64, i0 + f0:i0 + 512],
                       start=True, stop=True, tile_position=(0, 0))
                    mm(ps[:, 512 + f0:1024],
                       kt[pp][64:128, 128 * jj:128 * jj + 128],
                       qt[pp][64:128, i0 + f0:i0 + 512],
                       start=True, stop=True, tile_position=(64, 0))
                    e = p2s.tile([128, 1024], BF16, tag="e", name="e_s")
                    ev = e.rearrange("p (h x) -> p h x", x=512)
                    pv2 = ps.rearrange("p (h x) -> p h x", x=512)
                    nc.scalar.activation(ev[:, :, f0:512],
                                         pv2[:, :, f0:512], EXP,
                                         scale=SCALE)
                    if di >= 0:
                        nc.vector.tensor_mul(
                            ev[:, :, f0:f0 + 128],
                            ev[:, :, f0:f0 + 128],
                            mask_sb[:, None, :].to_broadcast((128, 2, 128)))
                    if pend is not None:
                        emit_av(*pend)
                    pend = (e, f0, jj)
                emit_av(*pend)
                stA = p2n.tile([65, 512], BF16, tag="stA", name="stA")
                stB = p2n.tile([65, 512], BF16, tag="stB", name="stB")
                nc.vector.tensor_copy(stA[:], poA[:])
                nc.vector.tensor_copy(stB[:], poB[:])
                for gg in (0, 1):
                    r0 = CR * (4 * gg + I)
                    nc.sync.dma_start(a2a_in[pp][r0:r0 + 64, :], stA[0:64, :])
                    nc.sync.dma_start(a2a_in[pp][r0 + 64:r0 + 128, :],
                                      stB[0:64, :])
                    nc.sync.dma_start(a2a_in[pp][r0 + 128:r0 + 129, :],
                                      stA[64:65, :])
                    nc.sync.dma_start(a2a_in[pp][r0 + 129:r0 + 130, :],
                                      stB[64:65, :])

            for Q in range(NI):
                # projections for token block Q (all 4 heads)
                for w, bcol, dst in (("wq", bqc, qt), ("wk", bkc, kt)):
                    for mt in (0, 1):
                        ps = qkp.tile([128, 512], F32, tag="pqk",
                                      name="ps_qk")
                        for kk in range(KT):
                            mm(ps[:],
                               w_sb[w][:, kk, 128 * mt:128 * mt + 128],
                               xt_sb[:, Q, kk, :],
                               start=(kk == 0), stop=(kk == KT - 1))
                        nc.vector.tensor_scalar_add(
                            dst[mt][:, 512 * Q:512 * Q + 512], ps[:],
                            bcol[:, mt:mt + 1])
                for t4 in range(4):
                    tt = 4 * Q + t4
                    ps = pvp.tile([128, HL * D], F32, tag="pv", name="ps_v")
                    for kk in range(KT):
                        mm(ps[:],
                           xt_sb[:, Q, kk, 128 * t4:128 * t4 + 128],
                           w_sb["wv"][:, kk, :],
                           start=(kk == 0), stop=(kk == KT - 1))
                    vv = vt[tt].rearrange("p (h x) -> p h x", x=VS)
                    nc.vector.tensor_add(vv[:, :, 0:D],
                                         ps.rearrange("p (h x) -> p h x",
                                                      x=D),
                                         bvb.rearrange("p (h x) -> p h x",
                                                       x=D))
                    nc.vector.tensor_copy(
                        vv[:, :, D:D + 1],
                        ones_sb.rearrange("p (h o) -> p h o", o=1))
                # pair-0 attention for query block Q rides this wave
                attention_block(0, Q)
            nc.gpsimd.collective_compute(
                "AllToAll", mybir.AluOpType.bypass,
                replica_groups=[list(range(N_CORES))],
                ins=[a2a_in[0].opt()], outs=[a2a_out[0].opt()])
            # wout lands during pair-1 attention, when input queues are
            # idle; split across all three queues so no single queue
            # carries the whole 2MB (the gpsimd queue later serves P3's
            # atf chunks)
            wout_dv = wout_d.rearrange("p (k c) -> p k c", k=KT)
            for i, eng in enumerate((nc.scalar, nc.sync, nc.gpsimd,
                                     nc.scalar)):
                eng.dma_start(wout_sb[:, 2 * i:2 * i + 2, :],
                              wout_dv[:, 2 * i:2 * i + 2, :])
            for Q in range(NI):
                attention_block(1, Q)
            nc.gpsimd.collective_compute(
                "AllToAll", mybir.AluOpType.bypass,
                replica_groups=[list(range(N_CORES))],
                ins=[a2a_in[1].opt()], outs=[a2a_out[1].opt()])
            for c in reversed(psum_ctx):
                c.__exit__(None, None, None)

            # ---------------- P3: output projection ----------------
            # Receiver-side softmax normalization, all on-chip: Z rows from
            # my group's 4 senders land as [8, 512] (multi-lane reciprocal),
            # then tiny ones-column matmuls broadcast 1/Z across the 64
            # head-dim partitions into PSUM, and the A^T tiles are scaled
            # straight from there — no DRAM round trip. The normalize chain
            # for pair 0 runs during the second AllToAll's transfer window.
            with (tc.tile_pool(name="p3s", bufs=2) as p3s,
                  tc.tile_pool(name="p3z", bufs=1, space="PSUM") as p3z,
                  tc.tile_pool(name="p3p", bufs=1, space="PSUM") as p3p):
                atns = []
                eng3 = [nc.sync, nc.scalar]

                def recv_chain(h):
                    """Fetch + normalize pair h's received A^T tiles,
                    pipelined per 128-row chunk: each chunk's DMA rides its
                    own queue and its normalize mul fires as it lands."""
                    av = a2a_out[h].rearrange("(G k p) c -> p G k c", k=4,
                                              p=CR)
                    zq = p3s.tile([8, 512], BF16, tag=f"zq{h}",
                                  name=f"zq{h}", bufs=1)
                    # element order (h2-major, then sender k, then token)
                    # matches zq's flat [8, 512] partition order h2*4+k
                    avz = a2a_out[h].rearrange("(G k p) c -> p k G c",
                                               k=4, p=CR)
                    nc.sync.dma_start(zq[:],
                                      avz[128:130, :, ds(gsel, 1), :])
                    zqf = p3s.tile([8, 512], F32, tag=f"zqf{h}",
                                   name=f"zqf{h}", bufs=1)
                    nc.vector.tensor_copy(zqf[:], zq[:])
                    rzq = p3s.tile([8, 512], F32, tag=f"rzq{h}",
                                   name=f"rzq{h}", bufs=1)
                    nc.vector.reciprocal(rzq[:], zqf[:])
                    rzqb = p3s.tile([8, 512], BF16, tag=f"rzqb{h}",
                                    name=f"rzqb{h}", bufs=1)
                    nc.vector.tensor_copy(rzqb[:], rzq[:])
                    rzp = p3z.tile([128, 4, 512], F32, tag="rzp",
                                   name=f"rzp{h}")
                    atf = p3s.tile([128, 4, 512], BF16, tag=f"atf{h}",
                                   name=f"atf{h}", bufs=1)
                    atn = p3s.tile([128, 4, 512], BF16, tag=f"atn{h}",
                                   name=f"atn{h}", bufs=1)
                    q3 = [nc.scalar, nc.gpsimd, nc.sync, nc.scalar]
                    for k4 in range(4):
                        q3[k4].dma_start(atf[:, k4, :],
                                         av[0:128, ds(gsel, 1), k4, :])
                        mm(rzp[:, k4, :], sel_sb[:, k4, :], rzqb[:],
                           start=True, stop=True)
                        nc.vector.tensor_mul(atn[:, k4, :], atf[:, k4, :],
                                             rzp[:, k4, :])
                    atns.append(atn)

                def pout_mms(pso, it, ct, kks):
                    for kk in kks:
                        h, k4 = divmod(kk, 4)
                        mm(pso[:],
                           atns[h][:, k4, 128 * it:128 * it + 128],
                           wout_sb[:, kk, 512 * ct:512 * ct + 512],
                           start=(kk == 0), stop=(kk == KT - 1))

                def pout_done(pso, it, ct):
                    osb = p3s.tile([128, 512], F32, tag="osb", name="osb")
                    nc.vector.tensor_add(
                        osb[:], pso[:], boutb[:, 512 * ct:512 * ct + 512])
                    eng3[it % 2].dma_start(
                        out_d[128 * it:128 * it + 128,
                              512 * ct:512 * ct + 512], osb[:])

                # pair-0 chunks arrive with the first AllToAll, so their
                # half of the contraction for the first two token tiles
                # runs during the second AllToAll's transfer window
                recv_chain(0)
                pouts = {}
                for it in (0, 1):
                    for ct in (0, 1):
                        pouts[(it, ct)] = p3p.tile(
                            [128, 512], F32, tag=f"po{it}{ct}",
                            name=f"po{it}{ct}", bufs=1)
                        pout_mms(pouts[(it, ct)], it, ct, range(4))
                recv_chain(1)
                # kk-major so each chunk's matmuls start as soon as that
                # chunk has landed and been normalized
                for kk in range(4, 8):
                    for it in (0, 1):
                        for ct in (0, 1):
                            pout_mms(pouts[(it, ct)], it, ct, [kk])
                for it in (0, 1):
                    for ct in (0, 1):
                        pout_done(pouts[(it, ct)], it, ct)
                for it in (2, 3):
                    for ct in (0, 1):
                        pso = p3p.tile([128, 512], F32,
                                       tag=f"po{it - 2}{ct}",
                                       name=f"po{it}{ct}", bufs=1)
                        pout_mms(pso, it, ct, range(8))
                        pout_done(pso, it, ct)


_NC_CACHE = {}

# test-only knobs: set TRACE=True before calling kernel() to profile; the
# BassKernelResults of the last run lands in LAST_RESULT. TRACE_CORES
# optionally selects which cores to profile (e.g. list(range(8))).
TRACE = False
TRACE_CORES = None
LAST_RESULT = None


def _get_nc():
    if "nc" not in _NC_CACHE:
        _NC_CACHE["nc"] = _build()
    return _NC_CACHE["nc"]


def kernel(x, Wq, bq, Wkv, bkv, Wout, bout):
    x = np.asarray(x, np.float32)
    Wq = np.asarray(Wq, np.float32)
    bq = np.asarray(bq, np.float32)
    Wkv = np.asarray(Wkv, np.float32)
    bkv = np.asarray(bkv, np.float32)
    Wout = np.asarray(Wout, np.float32)
    bout = np.asarray(bout, np.float32)

    def ktile(a):  # [128*KT_rows, width] -> [128, KT_rows*width], row-linear
        kk = a.shape[0] // 128
        return np.ascontiguousarray(
            a.reshape(kk, 128, a.shape[1]).transpose(1, 0, 2).reshape(128, -1))

    def xtile(a):  # x^T [1024, 2048] -> [128, (q kk 512)] token-major
        return np.ascontiguousarray(
            a.reshape(KT, 128, NI, 512).transpose(1, 2, 0, 3).reshape(128, -1))

    def tob(a):
        return np.ascontiguousarray(a.astype(bfloat16))

    mask = np.triu(np.ones((128, 128), np.float32))  # mask[p, c] = c >= p
    sel = np.zeros((8, 4, 128), np.float32)
    for p in range(128):
        for k in range(4):
            sel[(p // 64) * 4 + k, k, p] = 1.0
    sel = tob(sel.reshape(8, 512))
    xts = [tob(xtile(np.ascontiguousarray(x[g].T))) for g in range(B)]
    # out-proj contraction row order: collective h, rank kk carries heads
    # (4*kk + 2h, 4*kk + 2h + 1) -> permute Wout rows to match
    wout_perm = np.concatenate(
        [Wout[256 * kk + 128 * h:256 * kk + 128 * h + 128]
         for h in (0, 1) for kk in range(4)])
    wout_t = tob(ktile(wout_perm))
    in_maps = []
    for j in range(N_CORES):
        g, r = divmod(j, 4)
        cols = slice(HL * D * r, HL * D * (r + 1))
        in_maps.append({
            "xt": xts[g],
            "wq": tob(ktile(Wq[:, cols])),
            "wk": tob(ktile(Wkv[:, 0:DIM][:, cols])),
            "wv": tob(ktile(Wkv[:, DIM:2 * DIM][:, cols])),
            "wout": wout_t,
            "bq": np.ascontiguousarray(bq[cols][:, None]),
            "bk": np.ascontiguousarray(bkv[0:DIM][cols][:, None]),
            "bv": np.ascontiguousarray(bkv[DIM:2 * DIM][cols][None, :]),
            "bout": np.ascontiguousarray(bout[None, :]),
            "mask": tob(mask),
            "ones": np.ones((1, HL), bfloat16),
            "sel": sel,
        })
    res = run_bass_kernel_spmd(_get_nc(), in_maps, list(range(N_CORES)),
                               trace=TRACE, trace_cores=TRACE_CORES)
    global LAST_RESULT
    LAST_RESULT = res
    out = np.empty((B, N, DIM), np.float32)
    for j in range(N_CORES):
        g, r = divmod(j, 4)
        out[g, 512 * r:512 * (r + 1)] = res.results[j]["out"]
    return out


# revision 56
# speedup vs baseline: 1.0158x; 1.0102x over previous
"""Causal multi-head attention (b=2, n=2048, dim=1024, 16 heads) on 8 trn2
NeuronCores.

Sharding: core j = 4*g + r owns batch g and heads 4r..4r+3 (tensor parallel
over heads within each batch's 4-core group). Each core:
  P1  projects q/k (transposed layout [head_dim, tokens]) and v (natural
      [tokens, head_dim], ones-augmented) for its 4 heads from x^T. All
      matmul operands are bf16 (fp32 PSUM accumulate); rel-err budget is
      2e-2 so bf16 inputs are comfortably inside it and halve DMA + SBUF
      traffic while enabling fast-weight-load on the PE.
  P2  causal attention per head pair in S^T orientation, software-pipelined:
      slot j emits S^T(j) then AV(j-1), so the exp(j) on the scalar engine
      overlaps AV(j-1) + S^T(j+1) on the PE instead of stalling it.
      S^T = two row-group-concurrent matmuls (d=64 contraction each), exp
      without max subtraction (scores are O(1) here), triangular mask on
      diagonal tiles applied on gpsimd off the PE/ACT critical path,
      O'^T = V_aug.T @ expS^T accumulated in PSUM (row 64 = softmax
      denominator Z). Normalization reads PSUM directly: approx-fast
      reciprocal of the Z rows (51 ULP), DRAM-bounce broadcast, scale muls
      split across vector/gpsimd writing bf16 A2A staging.
  A2A transposes the sharding of A^T = [head_dim*heads, tokens] from
      head-sharded to token-sharded (8-core AllToAll on bf16 payload; each
      core addresses its group's chunks via partition_id-derived offsets).
  P3  out = A^T.T @ Wout for this core's 512-token block, plus biases.
Host: transposes x per batch, slices weights per head group, converts the
matmul path to bf16, gathers the 8 [512, 1024] row blocks into the full
[2, 2048, 1024] fp32 output.
"""
import numpy as np
from ml_dtypes import bfloat16

import concourse.bass as bass
import concourse.mybir as mybir
import concourse.tile as tile
from concourse.bass import AP, ds
from concourse.bass_utils import run_bass_kernel_spmd
from concourse.vector_clock import ScopedClock

F32 = mybir.dt.float32
BF16 = mybir.dt.bfloat16
EXP = mybir.ActivationFunctionType.Exp

N_CORES = 8
B, N, DIM, H = 2, 2048, 1024, 16
D = DIM // H                 # 64
HL = 4                       # heads per core
KT = DIM // 128              # 8 contraction k-tiles
NJ = N // 128                # 16 key tiles per batch
NI = N // 512                # 4 query i-blocks per batch
SCALE = float(D) ** -0.5


def _split_multi_waits(nc):
    """This walrus build rejects instructions carrying more than one sync
    wait. Hoist extra waits onto same-engine NoOps inserted directly before
    the offending instruction (engines execute their stream in order, so
    this preserves semantics)."""
    n = 0
    for f in nc.m.functions:
        for bb in f.blocks:
            insts = bb.instructions
            out = []
            changed = False
            for inst in insts:
                si = inst.sync_info
                waits = list(si.on_wait) if si is not None and si.on_wait else []
                if len(waits) > 1:
                    changed = True
                    for w in waits[:-1]:
                        nop = mybir.InstNoOp(name=f"I-waitfix-{n}", ins=[],
                                             outs=[])
                        n += 1
                        nop.engine = inst.engine
                        nop.sync_info = mybir.SyncInfo(on_wait=[w],
                                                       on_update=[])
                        out.append(nop)
                    si.on_wait = waits[-1:]
                out.append(inst)
            if changed:
                insts[:] = out
    return n


class _TC(tile.TileContext):
    """Tail drain in this walrus build only supports one sync-wait per CTRL
    instruction; spread the residual global-clock waits over SP nops, and
    split any remaining multi-wait instructions after scheduling."""

    def _drain_and_barrier(self, tick_clock, wait_clock):
        nop = self.nc.sync.nop()
        wait_clock.add_sem_waits(nop.ins, ScopedClock({None: tick_clock.global_clock}))
        si = nop.ins.sync_info
        waits = list(si.on_wait or []) if si is not None else []
        if len(waits) > 1:
            si.on_wait = waits[:1]
            for w in waits[1:]:
                extra = self.nc.sync.nop()
                extra.ins.sync_info = mybir.SyncInfo(on_wait=[w], on_update=[])
        self.nc.sync.drain()
        self.nc.all_engine_barrier()
        assert self.sems is not None
        popped = self.nc._tile_sem_poison_stack.pop()
        assert popped is self._sem_poison
        self.nc.clear_and_free_semaphores(list(self.sems.allocated().values()))
        self.nc.all_engine_barrier()

    def __exit__(self, exc_type, exc_val, exc_tb):
        r = super().__exit__(exc_type, exc_val, exc_tb)
        if exc_type is None:
            _split_multi_waits(self.nc)
        return r


def _bcast(src_dram_row, parts):
    """DRAM [1, n] row -> AP replicating it over `parts` partitions (step-0
    leading dim; only legal for DRAM sources)."""
    return AP(src_dram_row.tensor, src_dram_row.offset,
              [[0, parts]] + list(src_dram_row.ap)[1:])


def _build():
    nc = bass.Bass(trn_type="TRN2", target_bir_lowering=False, debug=False,
                   num_devices=N_CORES)
    dt = F32
    # pre-tiled on host: [128, KT*width] rows are fully linear so the bulk
    # DMAs run at line rate instead of 1KB-descriptor rate
    xt_d = nc.dram_tensor("xt", [128, KT * N], BF16, kind="ExternalInput").ap()
    wq_d = nc.dram_tensor("wq", [128, KT * HL * D], BF16, kind="ExternalInput").ap()
    wk_d = nc.dram_tensor("wk", [128, KT * HL * D], BF16, kind="ExternalInput").ap()
    wv_d = nc.dram_tensor("wv", [128, KT * HL * D], BF16, kind="ExternalInput").ap()
    wout_d = nc.dram_tensor("wout", [128, KT * DIM], BF16, kind="ExternalInput").ap()
    bq_d = nc.dram_tensor("bq", [HL * D, 1], dt, kind="ExternalInput").ap()
    bk_d = nc.dram_tensor("bk", [HL * D, 1], dt, kind="ExternalInput").ap()
    bv_d = nc.dram_tensor("bv", [1, HL * D], dt, kind="ExternalInput").ap()
    bout_d = nc.dram_tensor("bout", [1, DIM], dt, kind="ExternalInput").ap()
    mask_d = nc.dram_tensor("mask", [128, 128], BF16, kind="ExternalInput").ap()
    ones_d = nc.dram_tensor("ones", [1, HL], BF16, kind="ExternalInput").ap()
    sel_d = nc.dram_tensor("sel", [8, 4 * 128], BF16, kind="ExternalInput").ap()
    out_d = nc.dram_tensor("out", [N // HL, DIM], dt, kind="ExternalOutput").ap()

    with _TC(nc) as tc, \
            nc.allow_low_precision(reason="bf16 matmul operand staging"):
        _body(nc, tc, xt_d, wq_d, wk_d, wv_d, wout_d, bq_d, bk_d, bv_d,
              bout_d, mask_d, ones_d, sel_d, out_d)
    return nc


def _body(nc, tc, xt_d, wq_d, wk_d, wv_d, wout_d, bq_d, bk_d, bv_d, bout_d,
          mask_d, ones_d, sel_d, out_d):
    mm = nc.tensor.matmul
    with tc.tile_pool(name="persist", bufs=1) as pers:
        # Persistent SBUF: q^T/k^T per head pair, v (ones-augmented) per
        # 128-token tile, A^T per head pair, mask, biases.
        qt = [pers.tile([128, N], BF16, tag=f"qt{p}", name=f"qt{p}") for p in (0, 1)]
        kt = [pers.tile([128, N], BF16, tag=f"kt{p}", name=f"kt{p}") for p in (0, 1)]
        VS = 96  # per-head stride in v tiles: 192B keeps lhsT 128B-aligned
        vt = [pers.tile([128, HL * VS], BF16, tag=f"v{t}", name=f"v{t}")
              for t in range(NJ)]
        mask_sb = pers.tile([128, 128], BF16, tag="mask", name="mask_sb")
        bqc = pers.tile([128, 2], F32, tag="bqc", name="bqc")
        bkc = pers.tile([128, 2], F32, tag="bkc", name="bkc")
        bvb = pers.tile([128, HL * D], F32, tag="bvb", name="bvb")
        boutb = pers.tile([128, DIM], F32, tag="boutb", name="boutb")

        ones_sb = pers.tile([128, HL], BF16, tag="ones", name="ones_sb")
        # selector for the receiver-side 1/Z broadcast: rzp[:, k, :] =
        # sel[:, k, :].T @ rzq picks Z-row (p//64)*4+k for out partition p
        sel_sb = pers.tile([8, 4, 128], BF16, tag="sel", name="sel_sb")

        # -------- P1 + P2 pair 0, interleaved in causal waves --------
        # Input DMA is the startup bottleneck (3 dynamic queues at ~37 GB/s
        # each), so x is staged token-major and consumed in four 512-token
        # waves: wave Q projects q/k/v for its token block, then immediately
        # runs pair-0 attention for query block Q (causality means only
        # tokens <= 512(Q+1) are needed). The scalar engine's exp stream —
        # the P2 bottleneck — starts ~15us in instead of after all of P1.
        # multi-engine partition id so gsel-addressed DMAs can issue from
        # any DMA-capable queue, not just SP
        pid = nc.partition_id()
        gsel = nc.snap(pid // 4, min_val=0, max_val=1)
        with (tc.tile_pool(name="p1s", bufs=1) as p1s,
              tc.tile_pool(name="p3w", bufs=1) as p3w,
              tc.tile_pool(name="p2s", bufs=4) as p2s,
              tc.tile_pool(name="p2n", bufs=3) as p2n,
              tc.tile_pool(name="p3s", bufs=2) as p3s,
              tc.tile_pool(name="dram", bufs=1, space="DRAM") as dram):
            w_sb = {}
            for nm in ("wq", "wk", "wv"):
                w_sb[nm] = p1s.tile([128, KT, HL * D], BF16, tag=nm,
                                    name=f"{nm}_sb")
            xt_sb = p1s.tile([128, NI, KT, 512], BF16, tag="xt", name="xt_sb")
            wout_sb = p3w.tile([128, KT, DIM], BF16, tag="wout",
                               name="wout_sb")
            # bulk input plan, in consumption order, split into ~256-512KB
            # chunks spread over the three queues so wave 0's operands land
            # in ~7us and later waves stream in behind the compute
            xt_dv = xt_d.rearrange("p (q k n) -> p q k n", q=NI, k=KT)
            wq_dv = wq_d.rearrange("p (k e) -> p k e", k=KT)
            wk_dv = wk_d.rearrange("p (k e) -> p k e", k=KT)
            wv_dv = wv_d.rearrange("p (k e) -> p k e", k=KT)
            plan = [
                (nc.scalar, w_sb["wq"][:, 0:4, :], wq_dv[:, 0:4, :]),
                (nc.sync, xt_sb[:, 0, 0:4, :], xt_dv[:, 0, 0:4, :]),
                (nc.gpsimd, xt_sb[:, 0, 4:8, :], xt_dv[:, 0, 4:8, :]),
                (nc.scalar, w_sb["wq"][:, 4:8, :], wq_dv[:, 4:8, :]),
                (nc.sync, w_sb["wk"][:, 0:4, :], wk_dv[:, 0:4, :]),
                (nc.gpsimd, w_sb["wk"][:, 4:8, :], wk_dv[:, 4:8, :]),
                (nc.scalar, w_sb["wv"][:], wv_dv),
                (nc.sync, xt_sb[:, 1, 0:4, :], xt_dv[:, 1, 0:4, :]),
                (nc.gpsimd, xt_sb[:, 1, 4:8, :], xt_dv[:, 1, 4:8, :]),
                (nc.scalar, xt_sb[:, 2, 0:4, :], xt_dv[:, 2, 0:4, :]),
                (nc.sync, xt_sb[:, 2, 4:8, :], xt_dv[:, 2, 4:8, :]),
                (nc.gpsimd, xt_sb[:, 3, 0:4, :], xt_dv[:, 3, 0:4, :]),
                (nc.scalar, xt_sb[:, 3, 4:8, :], xt_dv[:, 3, 4:8, :]),
            ]
            for eng, dst_ap, src_ap in plan:
                eng.dma_start(dst_ap, src_ap)
            # tiny constants trail the bulk issues (the serial SWDGE issue
            # cost would otherwise delay wave 0's x chunk by ~8us); they
            # still land well before their first uses
            nc.gpsimd.dma_start(mask_sb[:], mask_d[:])
            nc.gpsimd.dma_start(bqc[:],
                                bq_d.rearrange("(m p) o -> p (m o)", p=128))
            nc.gpsimd.dma_start(bkc[:],
                                bk_d.rearrange("(m p) o -> p (m o)", p=128))
            nc.gpsimd.dma_start(bvb[:], _bcast(bv_d[0:1, :], 128))
            nc.gpsimd.dma_start(boutb[:], _bcast(bout_d[0:1, :], 128))
            nc.gpsimd.dma_start(ones_sb[:], _bcast(ones_d[0:1, :], 128))
            nc.gpsimd.dma_start(sel_sb[:],
                                sel_d.rearrange("p (k e) -> p k e", k=4))

            CR = 130
            a2a_in = [dram.tile([8 * CR, 512], BF16, name=f"a2a_in{h}")
                      for h in (0, 1)]
            a2a_out = [dram.tile([8 * CR, 512], BF16, name=f"a2a_out{h}")
                       for h in (0, 1)]
            # PSUM during the wave phase: qk 1 + v 1 + scores 4 + O' 2 = 8
            psum_ctx = (tc.tile_pool(name="qkp", bufs=1, space="PSUM"),
                        tc.tile_pool(name="pvp", bufs=1, space="PSUM"),
                        tc.tile_pool(name="sp", bufs=2, space="PSUM"),
                        tc.tile_pool(name="op", bufs=1, space="PSUM"))
            qkp, pvp, sp, op = [c.__enter__() for c in psum_ctx]

            def attention_block(pp, I):
                """Pair pp, query block I: software-pipelined S^T / exp /
                AV; ships unnormalized O' + Z rows to the A2A staging
                buffer (chunk = 130 rows; 8-core AllToAll with the
                4-core-group duplication trick)."""
                i0 = 512 * I
                last = 4 * I + 3
                poA = op.tile([D + 1, 512], F32, tag="oA", name="poA")
                poB = op.tile([D + 1, 512], F32, tag="oB", name="poB")

                def emit_av(e, f0, jj):
                    vv = vt[jj].rearrange("p (h x) -> p h x", x=VS)
                    mm(poA[:, f0:512], vv[:, 2 * pp, 0:D + 1],
                       e[:, f0:512],
                       start=(jj == 0), stop=(jj == last))
                    mm(poB[:, f0:512], vv[:, 2 * pp + 1, 0:D + 1],
                       e[:, 512 + f0:1024],
                       start=(jj == 0), stop=(jj == last))

                pend = None
                for jj in range(4 * I + 4):
                    di = jj - 4 * I
                    f0 = 128 * di if di >= 0 else 0
                    ps = sp.tile([128, 1024], F32, tag="s", name="ps_s")
                    # explicit row-group tile positions: the two heads'
                    # d=64-contraction matmuls occupy disjoint halves of
                    # the PE array and run concurrently
                    mm(ps[:, f0:512],
                       kt[pp][0:64, 128 * jj:128 * jj + 128],
                       qt[pp][0:64, i0 + f0:i0 + 512],
                       start=True, stop=True, tile_position=(0, 0))
                    mm(ps[:, 512 + f0:1024],
                       kt[pp][64:128, 128 * jj:128 * jj + 128],
                       qt[pp][64:128, i0 + f0:i0 + 512],
                       start=True, stop=True, tile_position=(64, 0))
                    e = p2s.tile([128, 1024], BF16, tag="e", name="e_s")
                    ev = e.rearrange("p (h x) -> p h x", x=512)
                    pv2 = ps.rearrange("p (h x) -> p h x", x=512)
                    nc.scalar.activation(ev[:, :, f0:512],
                                         pv2[:, :, f0:512], EXP,
                                         scale=SCALE)
                    if di >= 0:
                        nc.vector.tensor_mul(
                            ev[:, :, f0:f0 + 128],
                            ev[:, :, f0:f0 + 128],
                            mask_sb[:, None, :].to_broadcast((128, 2, 128)))
                    if pend is not None:
                        emit_av(*pend)
                    pend = (e, f0, jj)
                emit_av(*pend)
                stA = p2n.tile([65, 512], BF16, tag="stA", name="stA")
                stB = p2n.tile([65, 512], BF16, tag="stB", name="stB")
                nc.vector.tensor_copy(stA[:], poA[:])
                nc.vector.tensor_copy(stB[:], poB[:])
                for gg in (0, 1):
                    r0 = CR * (4 * gg + I)
                    nc.sync.dma_start(a2a_in[pp][r0:r0 + 64, :], stA[0:64, :])
                    nc.sync.dma_start(a2a_in[pp][r0 + 64:r0 + 128, :],
                                      stB[0:64, :])
                    nc.sync.dma_start(a2a_in[pp][r0 + 128:r0 + 129, :],
                                      stA[64:65, :])
                    nc.sync.dma_start(a2a_in[pp][r0 + 129:r0 + 130, :],
                                      stB[64:65, :])

            for Q in range(NI):
                # projections for token block Q (all 4 heads)
                for w, bcol, dst in (("wq", bqc, qt), ("wk", bkc, kt)):
                    for mt in (0, 1):
                        ps = qkp.tile([128, 512], F32, tag="pqk",
                                      name="ps_qk")
                        for kk in range(KT):
                            mm(ps[:],
                               w_sb[w][:, kk, 128 * mt:128 * mt + 128],
                               xt_sb[:, Q, kk, :],
                               start=(kk == 0), stop=(kk == KT - 1))
                        nc.vector.tensor_scalar_add(
                            dst[mt][:, 512 * Q:512 * Q + 512], ps[:],
                            bcol[:, mt:mt + 1])
                for t4 in range(4):
                    tt = 4 * Q + t4
                    ps = pvp.tile([128, HL * D], F32, tag="pv", name="ps_v")
                    for kk in range(KT):
                        mm(ps[:],
                           xt_sb[:, Q, kk, 128 * t4:128 * t4 + 128],
                           w_sb["wv"][:, kk, :],
                           start=(kk == 0), stop=(kk == KT - 1))
                    vv = vt[tt].rearrange("p (h x) -> p h x", x=VS)
                    nc.vector.tensor_add(vv[:, :, 0:D],
                                         ps.rearrange("p (h x) -> p h x",
                                                      x=D),
                                         bvb.rearrange("p (h x) -> p h x",
                                                       x=D))
                    nc.vector.tensor_copy(
                        vv[:, :, D:D + 1],
                        ones_sb.rearrange("p (h o) -> p h o", o=1))
                # pair-0 attention for query block Q rides this wave
                attention_block(0, Q)
            nc.gpsimd.collective_compute(
                "AllToAll", mybir.AluOpType.bypass,
                replica_groups=[list(range(N_CORES))],
                ins=[a2a_in[0].opt()], outs=[a2a_out[0].opt()])
            # wout lands during pair-1 attention, when input queues are
            # idle; split across all three queues so no single queue
            # carries the whole 2MB (the gpsimd queue later serves P3's
            # atf chunks)
            wout_dv = wout_d.rearrange("p (k c) -> p k c", k=KT)
            for i, eng in enumerate((nc.scalar, nc.sync, nc.gpsimd,
                                     nc.scalar)):
                eng.dma_start(wout_sb[:, 2 * i:2 * i + 2, :],
                              wout_dv[:, 2 * i:2 * i + 2, :])
            fetched = {}

            def fetch_chain(h):
                """SBUF-only part of the receive path: pull pair h's A2A
                output + Z rows and compute 1/Z. Emitted as early as the
                data can exist so the transfers and the reciprocal hide
                under attention / the other collective. No gpsimd queue
                use — the Pool stream is blocked by collective enqueues."""
                av = a2a_out[h].rearrange("(G k p) c -> p G k c", k=4, p=CR)
                avz = a2a_out[h].rearrange("(G k p) c -> p k G c", k=4, p=CR)
                zq = p3s.tile([8, 512], BF16, tag=f"zq{h}",
                              name=f"zq{h}", bufs=1)
                nc.sync.dma_start(zq[:], avz[128:130, :, ds(gsel, 1), :])
                zqf = p3s.tile([8, 512], F32, tag=f"zqf{h}",
                               name=f"zqf{h}", bufs=1)
                nc.vector.tensor_copy(zqf[:], zq[:])
                rzq = p3s.tile([8, 512], F32, tag=f"rzq{h}",
                               name=f"rzq{h}", bufs=1)
                nc.vector.reciprocal(rzq[:], zqf[:])
                rzqb = p3s.tile([8, 512], BF16, tag=f"rzqb{h}",
                                name=f"rzqb{h}", bufs=1)
                nc.vector.tensor_copy(rzqb[:], rzq[:])
                atf = p3s.tile([128, 4, 512], BF16, tag=f"atf{h}",
                               name=f"atf{h}", bufs=1)
                q3 = [nc.scalar, nc.sync, nc.scalar, nc.sync]
                for k4 in range(4):
                    q3[k4].dma_start(atf[:, k4, :],
                                     av[0:128, ds(gsel, 1), k4, :])
                fetched[h] = (rzqb, atf)

            for Q in range(NI):
                attention_block(1, Q)
                if Q == 0:
                    # pair-0's A2A has completed by now; fetch + 1/Z run
                    # under the remaining pair-1 attention
                    fetch_chain(0)
            nc.gpsimd.collective_compute(
                "AllToAll", mybir.AluOpType.bypass,
                replica_groups=[list(range(N_CORES))],
                ins=[a2a_in[1].opt()], outs=[a2a_out[1].opt()])
            fetch_chain(1)
            for c in reversed(psum_ctx):
                c.__exit__(None, None, None)

            # ---------------- P3: output projection ----------------
            # Receiver-side softmax normalization, all on-chip: Z rows from
            # my group's 4 senders land as [8, 512] (multi-lane reciprocal),
            # then tiny ones-column matmuls broadcast 1/Z across the 64
            # head-dim partitions into PSUM, and the A^T tiles are scaled
            # straight from there — no DRAM round trip. The normalize chain
            # for pair 0 runs during the second AllToAll's transfer window.
            with (tc.tile_pool(name="p3z", bufs=1, space="PSUM") as p3z,
                  tc.tile_pool(name="p3p", bufs=1, space="PSUM") as p3p):
                atns = []
                eng3 = [nc.sync, nc.scalar]

                def recv_chain(h):
                    """Normalize pair h's prefetched A^T tiles: broadcast
                    1/Z across partitions via selector matmuls into PSUM,
                    scale per 128-row chunk."""
                    rzqb, atf = fetched[h]
                    rzp = p3z.tile([128, 4, 512], F32, tag="rzp",
                                   name=f"rzp{h}")
                    atn = p3s.tile([128, 4, 512], BF16, tag=f"atn{h}",
                                   name=f"atn{h}", bufs=1)
                    for k4 in range(4):
                        mm(rzp[:, k4, :], sel_sb[:, k4, :], rzqb[:],
                           start=True, stop=True)
                        nc.vector.tensor_mul(atn[:, k4, :], atf[:, k4, :],
                                             rzp[:, k4, :])
                    atns.append(atn)

                def pout_mms(pso, it, ct, kks):
                    for kk in kks:
                        h, k4 = divmod(kk, 4)
                        mm(pso[:],
                           atns[h][:, k4, 128 * it:128 * it + 128],
                           wout_sb[:, kk, 512 * ct:512 * ct + 512],
                           start=(kk == 0), stop=(kk == KT - 1))

                def pout_done(pso, it, ct):
                    osb = p3s.tile([128, 512], F32, tag="osb", name="osb")
                    nc.vector.tensor_add(
                        osb[:], pso[:], boutb[:, 512 * ct:512 * ct + 512])
                    eng3[it % 2].dma_start(
                        out_d[128 * it:128 * it + 128,
                              512 * ct:512 * ct + 512], osb[:])

                # pair-0 chunks arrive with the first AllToAll, so their
                # half of the contraction for the first two token tiles
                # runs during the second AllToAll's transfer window
                recv_chain(0)
                pouts = {}
                for it in (0, 1):
                    for ct in (0, 1):
                        pouts[(it, ct)] = p3p.tile(
                            [128, 512], F32, tag=f"po{it}{ct}",
                            name=f"po{it}{ct}", bufs=1)
                        pout_mms(pouts[(it, ct)], it, ct, range(4))
                recv_chain(1)
                # kk-major so each chunk's matmuls start as soon as that
                # chunk has landed and been normalized
                for kk in range(4, 8):
                    for it in (0, 1):
                        for ct in (0, 1):
                            pout_mms(pouts[(it, ct)], it, ct, [kk])
                for it in (0, 1):
                    for ct in (0, 1):
                        pout_done(pouts[(it, ct)], it, ct)
                for it in (2, 3):
                    for ct in (0, 1):
                        pso = p3p.tile([128, 512], F32,
                                       tag=f"po{it - 2}{ct}",
                                       name=f"po{it}{ct}", bufs=1)
                        pout_mms(pso, it, ct, range(8))
                        pout_done(pso, it, ct)


_NC_CACHE = {}

# test-only knobs: set TRACE=True before calling kernel() to profile; the
# BassKernelResults of the last run lands in LAST_RESULT. TRACE_CORES
# optionally selects which cores to profile (e.g. list(range(8))).
TRACE = False
TRACE_CORES = None
LAST_RESULT = None


def _get_nc():
    if "nc" not in _NC_CACHE:
        _NC_CACHE["nc"] = _build()
    return _NC_CACHE["nc"]


def kernel(x, Wq, bq, Wkv, bkv, Wout, bout):
    x = np.asarray(x, np.float32)
    Wq = np.asarray(Wq, np.float32)
    bq = np.asarray(bq, np.float32)
    Wkv = np.asarray(Wkv, np.float32)
    bkv = np.asarray(bkv, np.float32)
    Wout = np.asarray(Wout, np.float32)
    bout = np.asarray(bout, np.float32)

    def ktile(a):  # [128*KT_rows, width] -> [128, KT_rows*width], row-linear
        kk = a.shape[0] // 128
        return np.ascontiguousarray(
            a.reshape(kk, 128, a.shape[1]).transpose(1, 0, 2).reshape(128, -1))

    def xtile(a):  # x^T [1024, 2048] -> [128, (q kk 512)] token-major
        return np.ascontiguousarray(
            a.reshape(KT, 128, NI, 512).transpose(1, 2, 0, 3).reshape(128, -1))

    def tob(a):
        return np.ascontiguousarray(a.astype(bfloat16))

    mask = np.triu(np.ones((128, 128), np.float32))  # mask[p, c] = c >= p
    sel = np.zeros((8, 4, 128), np.float32)
    for p in range(128):
        for k in range(4):
            sel[(p // 64) * 4 + k, k, p] = 1.0
    sel = tob(sel.reshape(8, 512))
    xts = [tob(xtile(np.ascontiguousarray(x[g].T))) for g in range(B)]
    # out-proj contraction row order: collective h, rank kk carries heads
    # (4*kk + 2h, 4*kk + 2h + 1) -> permute Wout rows to match
    wout_perm = np.concatenate(
        [Wout[256 * kk + 128 * h:256 * kk + 128 * h + 128]
         for h in (0, 1) for kk in range(4)])
    wout_t = tob(ktile(wout_perm))
    in_maps = []
    for j in range(N_CORES):
        g, r = divmod(j, 4)
        cols = slice(HL * D * r, HL * D * (r + 1))
        in_maps.append({
            "xt": xts[g],
            "wq": tob(ktile(Wq[:, cols])),
            "wk": tob(ktile(Wkv[:, 0:DIM][:, cols])),
            "wv": tob(ktile(Wkv[:, DIM:2 * DIM][:, cols])),
            "wout": wout_t,
            "bq": np.ascontiguousarray(bq[cols][:, None]),
            "bk": np.ascontiguousarray(bkv[0:DIM][cols][:, None]),
            "bv": np.ascontiguousarray(bkv[DIM:2 * DIM][cols][None, :]),
            "bout": np.ascontiguousarray(bout[None, :]),
            "mask": tob(mask),
            "ones": np.ones((1, HL), bfloat16),
            "sel": sel,
        })
    res = run_bass_kernel_spmd(_get_nc(), in_maps, list(range(N_CORES)),
                               trace=TRACE, trace_cores=TRACE_CORES)
    global LAST_RESULT
    LAST_RESULT = res
    out = np.empty((B, N, DIM), np.float32)
    for j in range(N_CORES):
        g, r = divmod(j, 4)
        out[g, 512 * r:512 * (r + 1)] = res.results[j]["out"]
    return out
